# revision 4
# baseline (speedup 1.0000x reference)
"""GroupContrastLoss on 8 trn2 NeuronCores via Bass/Tile.

Math (reference):
  pos   = (gt == 1)                              [B,K,H,W]
  fnorm = feat / max(||feat||_C, eps)            per-pixel L2 over C
  k0    = einsum('bkhw,bchw->kc', pos, fnorm)    [K,C]   (global sum!)
  k0n   = k0 / max(||k0||_C, eps)
  logits= einsum('kc,bchw->bkhw', k0n, fnorm)/tau
  loss  = -sum(pos * log_softmax(logits, k)) / sum(pos)

Key identity used here: sum_pix pos*logits = <k0n/tau, k0_local> summed over
cores, since k0_local is by definition sum_pix pos*invr*feat.  This removes
the whole pos*z pixel path from phase 2.

Sharding: pixels (b, hw) split into 8 contiguous shards (2 per batch image,
32768 pixels each).  Per-core partial k0 [19,256] is AllReduced on-device
(Shared-output mesh) between the two phases; scalar partials
(sum npos*lse, num_pos, <k0ns,k0_loc>) are combined on host.

Phase 1: feat is read from HBM once as raw f32 via HWDGE DMA (fast path; the
SWDGE casting DMA only sustains ~10GB/s/engine with poor concurrency), cast
to bf16 on DVE/ACT/GPSIMD into a persistent channel-major stash
fa16[128, NT, 16, 2, 128] whose free layout interleaves (pix_hi, chan_half,
pix_lo) so the per-tile xbar transpose yields pixel-major tiles with the 256
channels CONTIGUOUS per pixel (fast SoS ops).  Per-tile transposes alternate
between the two HWDGE queues (sync/scalar) because DMA_TRANSPOSE blocks its
issuing queue for the transfer duration.  Per-pixel inv-norms fold into the
transposed gt mask (posw) for the k0 accumulation (16 bf16 matmuls/tile).

Between phases, a chain of dummy matmuls keeps the PE HAM-warm across the
collective gap so phase-2 matmuls run at 2.4GHz instead of 1.2GHz.

Phase 2 computes logits as [K,512] bf16 matmuls (k0nT stationary), stages
PSUM->SBUF as bf16 K-major in the gt16 buffers, one xbar transpose per pair
back to pixel-major, then exp/sum softmax denominator only.  Ln and the loss
combination happen in one batched tail (EXP and LN live in different hw
activation tables; per-tile Ln would thrash table loads).
"""

import numpy as np

TAU = 0.07
EPS = 1e-12
B, C, H, W, K = 4, 256, 256, 256, 19
HW = H * W
NCORES = 8
SHARD = B * HW // NCORES        # 32768 pixels per core
TILE_PIX = 2048                 # pixels per tile iteration
NCH = TILE_PIX // 128           # 16 chunks of 128 pixels
NT = SHARD // TILE_PIX          # 16 tile iterations
NP = NT // 2                    # 8 tile pairs
CH = C // 2                     # 128, feat channel half
KP = 32                         # classes padded to 32 for DMA transpose
GP = 512                        # pixels per logit matmul group
NG = TILE_PIX // GP             # 4 groups per tile
HPIX = TILE_PIX // 2            # 1024, pixels per f32 load half
NWARM = 170                     # dummy matmuls bridging the collective gap

_CACHE = {}


def _build_nc(ncores=NCORES):
    import concourse.bass as bass
    import concourse.bass_isa as bass_isa
    import concourse.bacc as bacc
    import concourse.mybir as mybir
    from concourse import tile, masks

    f32 = mybir.dt.float32
    bf16 = mybir.dt.bfloat16
    AX = mybir.AxisListType
    AF = mybir.ActivationFunctionType
    ALU = mybir.AluOpType

    nc = bacc.Bacc("TRN2", target_bir_lowering=False, debug=False,
                   num_devices=ncores)

    feat_in = nc.dram_tensor("feat_s", [C, SHARD], f32, kind="ExternalInput")
    gt_in = nc.dram_tensor("gt_s", [K, SHARD], f32, kind="ExternalInput")
    out_part = nc.dram_tensor("part", [1, 3], f32, kind="ExternalOutput")

    with tile.TileContext(nc) as tc:
        with (
            tc.tile_pool(name="persist", bufs=1) as pp,
            tc.tile_pool(name="fstage", bufs=2) as pfs,
            tc.tile_pool(name="ft", bufs=2) as pft,
            tc.tile_pool(name="small", bufs=2) as ps,
            tc.tile_pool(name="p2", bufs=1) as p2,
            tc.tile_pool(name="dram", bufs=1, space="DRAM") as pd,
        ):
            # persistent stashes
            # channel-major bf16 feat; free layout (tile, pix_hi, chan_half,
            # pix_lo) so per-pixel channels come out contiguous after xbar
            fa16 = pp.tile([128, NT, NCH, 2, 128], bf16)
            posT16 = pp.tile([128, NT * NCH, KP], bf16)   # gt transposed
            invr_all = pp.tile([128, NT * NCH], f32)
            npos_cols = pp.tile([128, NT * NCH], f32)
            s_all = pp.tile([128, NT * NCH], f32)
            sq_v = pp.tile([128, C], bf16)                # SoS scratch outs
            sq_s = pp.tile([128, C], bf16)
            wsrc = pp.tile([128, GP], bf16)               # warmup mm source
            lred = pp.tile([128, 4], f32)                 # loss partials

            nc.vector.memset(wsrc[:], 0.0)
            nc.gpsimd.memset(lred[:], 0.0)

            # pair-sized K-major staging, shared by phase 1 (gt) and
            # phase 2 (z); rows K:KP zeroed once, never rewritten
            NGT = 2
            gt16 = [pp.tile([KP, 2 * TILE_PIX], bf16, name=f"gt16_{i}")
                    for i in range(NGT)]
            nc.vector.memset(gt16[0][:], 0.0)
            nc.gpsimd.memset(gt16[1][:], 0.0)

            # ---------------- phase 1: k0 accumulation ----------------
            with tc.tile_pool(name="psA", bufs=1, space="PSUM") as psA:
                k0_ps = psA.tile([K, C], f32)

                fstq = {}
                ftps = {}

                def _load(t):
                    # raw f32 halves via HWDGE, one per queue
                    tiles = []
                    for hh in range(2):
                        sl = slice(t * TILE_PIX + hh * HPIX,
                                   t * TILE_PIX + (hh + 1) * HPIX)
                        fst = pfs.tile([128, 2, HPIX], f32, tag="fst",
                                       name=f"fst_{t}_{hh}")
                        src = feat_in[:, sl].rearrange("(h p) x -> p h x",
                                                       h=2)
                        eng = nc.sync if hh == 0 else nc.scalar
                        eng.dma_start(fst[:], src)
                        tiles.append(fst)
                    fstq[t] = tiles

                def _cast(t):
                    f0, f1 = fstq.pop(t)
                    for hh, fst in ((0, f0), (1, f1)):
                        base = hh * (NCH // 2)
                        for eng, p0, p1 in ((nc.vector, 0, 3),
                                            (nc.scalar, 3, 6),
                                            (nc.gpsimd, 6, 8)):
                            out_ap = fa16[:, t, base + p0:base + p1, :, :]
                            in_ap = fst[:, :, p0 * 128:p1 * 128].rearrange(
                                "p h (f x) -> p f h x", x=128)
                            if eng is nc.scalar:
                                eng.copy(out_ap, in_ap)
                            else:
                                eng.tensor_copy(out_ap, in_ap)

                def _transpose(t):
                    ftp = pft.tile([128, NCH, 2, 128], bf16, tag="ftp",
                                   name=f"ftp_{t}")
                    eng = nc.sync if t % 2 == 0 else nc.scalar
                    eng.dma_start(ftp[:], fa16[:, t, :, :, :], transpose=True)
                    ftps[t] = ftp

                def _gt_pair(pr):
                    psl = slice(2 * pr * TILE_PIX, (2 * pr + 2) * TILE_PIX)
                    nc.gpsimd.dma_start(gt16[pr % NGT][0:K, :],
                                        gt_in[:, psl])

                def _gt_transpose(pr):
                    ptsl = slice(2 * pr * NCH, (2 * pr + 2) * NCH)
                    eng = nc.scalar if pr % 2 == 0 else nc.sync
                    eng.dma_start(posT16[:, ptsl, :], gt16[pr % NGT][:],
                                  transpose=True)

                # prologue: 2 tiles deep
                _load(0)
                _gt_pair(0)
                _load(1)
                _gt_pair(1)
                _cast(0)
                _transpose(0)
                _cast(1)
                _transpose(1)

                NDV = 9   # SoS chunks on DVE; rest on Scalar
                for t in range(NT):
                    if t + 2 < NT:
                        _load(t + 2)
                    if t % 2 == 0:
                        _gt_transpose(t // 2)
                        if t // 2 + 2 < NP:
                            _gt_pair(t // 2 + 2)
                    if t + 2 < NT:
                        _cast(t + 2)
                        _transpose(t + 2)

                    ftp = ftps.pop(t)
                    tsl = slice(t * NCH, (t + 1) * NCH)
                    ss = ps.tile([128, NCH], f32, tag="ss", name=f"ss_{t}")
                    for j in range(NCH):
                        src = ftp[:, j, :, :]
                        if j < NDV:
                            nc.vector.scalar_tensor_tensor(
                                out=sq_v[:], in0=src, scalar=1.0,
                                in1=src, op0=ALU.mult, op1=ALU.mult,
                                accum_out=ss[:, j:j + 1])
                        else:
                            nc.scalar.activation(
                                sq_s[:], src, AF.Square,
                                accum_out=ss[:, j:j + 1])
                    srt = ps.tile([128, NCH], f32, tag="srt",
                                  name=f"srt_{t}")
                    nc.scalar.sqrt(srt[:], ss[:])
                    nc.vector.reciprocal(invr_all[:, tsl], srt[:])

                    posw = ps.tile([128, NCH, K], bf16, tag="posw",
                                   name=f"posw_{t}")
                    nc.vector.tensor_mul(
                        posw[:], posT16[:, tsl, 0:K],
                        invr_all[:, tsl].unsqueeze(2).broadcast_to(
                            [128, NCH, K]))
                    nc.vector.tensor_reduce(
                        npos_cols[:, tsl], posT16[:, tsl, 0:K],
                        axis=AX.X, op=ALU.add)

                    for j in range(NCH):
                        nc.tensor.matmul(
                            k0_ps[:], posw[:, j, :],
                            ftp[:, j, :, :],
                            start=(t == 0 and j == 0),
                            stop=(t == NT - 1 and j == NCH - 1),
                            skip_group_check=True)

                k0_sb = pp.tile([K, C], f32)
                nc.scalar.copy(k0_sb[:], k0_ps[:])

                # dummy matmuls to keep PE HAM-warm across the collective
                # gap (write garbage into k0_ps, already copied out)
                for w in range(NWARM):
                    nc.tensor.matmul(k0_ps[:], wsrc[:, 0:K], wsrc[:, 0:C],
                                     start=True, stop=True,
                                     skip_group_check=True)

            # ---------------- AllReduce k0 across 8 cores ----------------
            k0_loc = pd.tile([K, C], f32)
            k0_sum = pd.tile([K, C], f32, addr_space="Shared")
            nc.sync.dma_start(k0_loc[:], k0_sb[:])
            nc.gpsimd.collective_compute(
                "AllReduce", ALU.add,
                ins=[k0_loc.opt()],
                outs=[k0_sum.opt()],
                replica_groups=[list(range(ncores))],
            )
            k0t = pp.tile([K, C], f32)
            nc.sync.dma_start(k0t[:], k0_sum[:])

            # k0ns = (k0 / max(||k0||, eps)) / tau, transposed to [c, 2, K]
            k0sq = pp.tile([K, C], f32)
            ssk = pp.tile([K, 1], f32)
            nc.scalar.activation(k0sq[:], k0t[:], AF.Square, accum_out=ssk[:])
            sk = pp.tile([K, 1], f32)
            nc.scalar.sqrt(sk[:], ssk[:])
            skm = pp.tile([K, 1], f32)
            nc.vector.tensor_scalar_max(skm[:], sk[:], EPS)
            invk = pp.tile([K, 1], f32)
            nc.vector.reciprocal(invk[:], skm[:])
            invks = pp.tile([K, 1], f32)
            nc.scalar.mul(invks[:], invk[:], 1.0 / TAU)
            # bf16 k0ns staged in the zero-padded 32-row tile, one tiny xbar
            # transpose gives k0n^T [c, 2, K] without touching PE/PSUM
            nc.vector.tensor_scalar_mul(gt16[0][0:K, 0:C], k0t[:], invks[:])
            k0nT16 = pp.tile([128, 2, KP], bf16)
            nc.sync.dma_start(k0nT16[:], gt16[0][:, 0:C], transpose=True)

            # dot partial: sum pos*z over this core's pixels
            #   = sum_kc k0ns[k,c] * k0_local[k,c]
            dotm = pp.tile([K, C], f32)
            nc.gpsimd.tensor_mul(dotm[:], k0t[:], k0_sb[:])
            dvec = pp.tile([K, 1], f32)
            nc.vector.reduce_sum(dvec[:], dotm[:], axis=AX.X)
            nc.vector.tensor_mul(lred[0:K, 2:3], dvec[:], invks[:])

            # ---------------- phase 2: logits, softmax denom, loss --------
            with tc.tile_pool(name="psB", bufs=2, space="PSUM") as psB:
                for t in range(NT):
                    pr, tt = t // 2, t % 2
                    lgA = psB.tile([K, 2, GP], f32, tag="lgA",
                                   name=f"lgA_{t}")
                    lgB = psB.tile([K, 2, GP], f32, tag="lgB",
                                   name=f"lgB_{t}")
                    lgs = [lgA[:, 0, :], lgA[:, 1, :], lgB[:, 0, :],
                           lgB[:, 1, :]]
                    for g in range(NG):
                        nc.tensor.matmul(
                            lgs[g], k0nT16[:, 0, 0:K],
                            fa16[:, t, 4 * g:4 * g + 4, 0, :],
                            start=True, stop=False, skip_group_check=True)
                    for g in range(NG):
                        nc.tensor.matmul(
                            lgs[g], k0nT16[:, 1, 0:K],
                            fa16[:, t, 4 * g:4 * g + 4, 1, :],
                            start=False, stop=True, skip_group_check=True)

                    # PSUM -> K-major bf16 staging (per tile), one xbar
                    # transpose + batched softmax denominator per pair
                    zs = gt16[pr % NGT]
                    zo = tt * TILE_PIX
                    nc.scalar.copy(zs[0:K, zo:zo + 2 * GP], lgA[:])
                    nc.vector.tensor_copy(zs[0:K, zo + 2 * GP:zo + 4 * GP],
                                          lgB[:])
                    if tt == 0:
                        continue
                    ptsl = slice(2 * pr * NCH, (2 * pr + 2) * NCH)
                    zT = p2.tile([128, 2 * NCH, KP], bf16, tag="zT",
                                 name=f"zT_{pr}")
                    teng = nc.sync if pr % 2 == 0 else nc.scalar
                    teng.dma_start(zT[:], zs[:], transpose=True)

                    ib = invr_all[:, ptsl].unsqueeze(2).broadcast_to(
                        [128, 2 * NCH, K])
                    y = p2.tile([128, 2 * NCH, K], f32, tag="y",
                                name=f"y_{pr}")
                    nc.gpsimd.tensor_mul(y[:], zT[:, :, 0:K], ib)
                    nc.scalar.activation(y[:], y[:], AF.Exp)
                    nc.vector.reduce_sum(s_all[:, ptsl], y[:], axis=AX.X)

                # deferred loss tail, batched over all 256 columns
                nc.scalar.activation(s_all[:], s_all[:], AF.Ln)
                nc.vector.reduce_sum(lred[:, 1:2], npos_cols[:], axis=AX.X)
                nc.vector.tensor_mul(npos_cols[:], npos_cols[:], s_all[:])
                nc.vector.reduce_sum(lred[:, 0:1], npos_cols[:], axis=AX.X)

                lfin = pp.tile([128, 4], f32)
                nc.gpsimd.partition_all_reduce(
                    lfin[:, 0:3], lred[:, 0:3], channels=128,
                    reduce_op=bass_isa.ReduceOp.add)
                nc.sync.dma_start(out_part[:], lfin[0:1, 0:3])

    nc.compile()
    return nc


def kernel(feat: np.ndarray, gt: np.ndarray) -> np.ndarray:
    from concourse.bass_utils import run_bass_kernel_spmd

    if "nc" not in _CACHE:
        _CACHE["nc"] = _build_nc()
    nc = _CACHE["nc"]

    feat_r = np.ascontiguousarray(feat, dtype=np.float32).reshape(B, C, HW)
    gt_r = np.ascontiguousarray(gt, dtype=np.float32).reshape(B, K, HW)
    per_batch = NCORES // B                       # 2 shards per image
    span = HW // per_batch                        # 32768
    in_maps = []
    for m in range(NCORES):
        b, lo = m // per_batch, (m % per_batch) * span
        in_maps.append({
            "feat_s": np.ascontiguousarray(feat_r[b, :, lo:lo + span]),
            "gt_s": np.ascontiguousarray(gt_r[b, :, lo:lo + span]),
        })

    res = run_bass_kernel_spmd(nc, in_maps, list(range(NCORES)))
    _CACHE["last_results"] = res
    parts = np.stack([r["part"].reshape(3) for r in res.results])
    nll_sum = float(np.sum(parts[:, 0].astype(np.float64)))
    num_pos = float(np.sum(parts[:, 1].astype(np.float64)))
    dot_sum = float(np.sum(parts[:, 2].astype(np.float64)))
    return np.asarray((nll_sum - dot_sum) / num_pos, dtype=np.float32)


# revision 6
# speedup vs baseline: 1.1007x; 1.1007x over previous
"""GroupContrastLoss on 8 trn2 NeuronCores via Bass/Tile.

Math (reference):
  pos   = (gt == 1)                              [B,K,H,W]
  fnorm = feat / max(||feat||_C, eps)            per-pixel L2 over C
  k0    = einsum('bkhw,bchw->kc', pos, fnorm)    [K,C]   (global sum!)
  k0n   = k0 / max(||k0||_C, eps)
  logits= einsum('kc,bchw->bkhw', k0n, fnorm)/tau
  loss  = -sum(pos * log_softmax(logits, k)) / sum(pos)

Key identity: sum_pix pos*logits = <k0n/tau, k0_local> per core, since
k0_local is by definition sum_pix pos*invr*feat.  This removes the whole
pos*z pixel path from phase 2 (no posT16 read, no second reduce).

Sharding: pixels (b, hw) split into 8 contiguous shards (2 per batch image,
32768 pixels each).  Per-core partial k0 [19,256] is AllReduced on-device
between the phases; scalar partials (sum npos*lse, num_pos, <k0ns,k0_loc>)
are combined on host.

Phase 1: feat is read from HBM exactly once via gpsimd casting DMAs (f32 in
DRAM -> bf16 in SBUF) into a channel-half-OUTER stash fa16[128, NP, 2, 2, 2048]
so each cast descriptor is one contiguous 8KB run per partition (128 descs
per dma_start instead of 256 -- Q7 descriptor generation was the cast-path
concurrency limiter).  Pixel-major feat/gt come from per-pair 2MB xbar DMA
transposes; DMA_TRANSPOSE blocks its issuing queue for the transfer
duration, so transposes alternate between the two HWDGE queues
(sync/scalar).  Per-pixel inv-norms come from chunked fused square+
accumulate ops (DVE scalar_tensor_tensor + Scalar Square+accum_out); invr
folds into the transposed gt mask (posw) for the k0 accumulation (16 bf16
matmuls per tile, PE otherwise free in phase 1).

Between phases a chain of dummy matmuls keeps the PE HAM-warm across the
collective gap so phase-2 matmuls run at 2.4GHz instead of 1.2GHz.

Phase 2 computes logits as [K,512] bf16 matmuls (k0nT stationary) straight
from the contiguous fa16 slices, stages PSUM->SBUF as bf16 K-major in the
gt16 buffers, one xbar transpose per pair back to pixel-major, then only
the exp/sum softmax denominator.  Ln and the loss combination happen in one
batched tail (EXP and LN live in different hw activation tables; per-tile
Ln would thrash table loads).
"""

import numpy as np

TAU = 0.07
EPS = 1e-12
B, C, H, W, K = 4, 256, 256, 256, 19
HW = H * W
NCORES = 8
SHARD = B * HW // NCORES        # 32768 pixels per core
TILE_PIX = 2048                 # pixels per tile iteration
NCH = TILE_PIX // 128           # 16 chunks of 128 pixels
NT = SHARD // TILE_PIX          # 16 tile iterations
NP = NT // 2                    # 8 tile pairs
CH = C // 2                     # 128, feat channel half
KP = 32                         # classes padded to 32 for DMA transpose
GP = 512                        # pixels per logit matmul group
NG = TILE_PIX // GP             # 4 groups per tile
NWARM = 450                     # dummy matmuls bridging the collective gap

_CACHE = {}


def _build_nc(ncores=NCORES):
    import concourse.bass as bass
    import concourse.bass_isa as bass_isa
    import concourse.bacc as bacc
    import concourse.mybir as mybir
    from concourse import tile, masks

    f32 = mybir.dt.float32
    bf16 = mybir.dt.bfloat16
    AX = mybir.AxisListType
    AF = mybir.ActivationFunctionType
    ALU = mybir.AluOpType

    nc = bacc.Bacc("TRN2", target_bir_lowering=False, debug=False,
                   num_devices=ncores)

    feat_in = nc.dram_tensor("feat_s", [C, SHARD], f32, kind="ExternalInput")
    gt_in = nc.dram_tensor("gt_s", [K, SHARD], f32, kind="ExternalInput")
    out_part = nc.dram_tensor("part", [1, 3], f32, kind="ExternalOutput")

    with tile.TileContext(nc) as tc:
        with (
            tc.tile_pool(name="persist", bufs=1) as pp,
            tc.tile_pool(name="ft", bufs=2) as pft,
            tc.tile_pool(name="small", bufs=2) as ps,
            tc.tile_pool(name="p2", bufs=1) as p2,
            tc.tile_pool(name="dram", bufs=1, space="DRAM") as pd,
        ):
            # persistent stashes
            # channel-major bf16 feat, channel-half OUTERMOST so each cast
            # DMA's per-partition run is one contiguous 8KB block
            fa16 = pp.tile([128, NP, 2, 2, TILE_PIX], bf16)
            posT16 = pp.tile([128, NT * NCH, KP], bf16)   # gt transposed
            invr_all = pp.tile([128, NT * NCH], f32)
            npos_cols = pp.tile([128, NT * NCH], f32)
            s_all = pp.tile([128, NT * NCH], f32)
            sq_v = pp.tile([128, C], bf16)                # SoS scratch outs
            sq_s = pp.tile([128, C], bf16)
            wsrc = pp.tile([128, GP], bf16)               # warmup mm source
            lred = pp.tile([128, 4], f32)                 # loss partials

            nc.vector.memset(wsrc[:], 0.0)
            nc.gpsimd.memset(lred[:], 0.0)

            # pair-sized K-major staging, shared by phase 1 (gt) and
            # phase 2 (z); rows K:KP zeroed once, never rewritten
            NGT = 2
            gt16 = [pp.tile([KP, 2 * TILE_PIX], bf16, name=f"gt16_{i}")
                    for i in range(NGT)]
            nc.vector.memset(gt16[0][:], 0.0)
            nc.gpsimd.memset(gt16[1][:], 0.0)

            # ---------------- phase 1: k0 accumulation ----------------
            with tc.tile_pool(name="psA", bufs=1, space="PSUM") as psA:
                k0_ps = psA.tile([K, C], f32)
                ftps = {}

                def _cast_pair(pr):
                    psl = slice(2 * pr * TILE_PIX, (2 * pr + 2) * TILE_PIX)
                    nc.gpsimd.dma_start(gt16[pr % NGT][0:K, :],
                                        gt_in[:, psl])
                    nc.gpsimd.dma_start(fa16[:, pr, 0, :, :],
                                        feat_in[0:CH, psl])
                    nc.gpsimd.dma_start(fa16[:, pr, 1, :, :],
                                        feat_in[CH:C, psl])

                def _transposes(pr):
                    ptsl = slice(2 * pr * NCH, (2 * pr + 2) * NCH)
                    ftp2 = pft.tile([128, 2, 2, NCH, 128], bf16, tag="ftp",
                                    name=f"ftp_{pr}")
                    feng = nc.sync if pr % 2 == 0 else nc.scalar
                    geng = nc.scalar if pr % 2 == 0 else nc.sync
                    feng.dma_start(ftp2[:], fa16[:, pr, :, :, :],
                                   transpose=True)
                    geng.dma_start(posT16[:, ptsl, :], gt16[pr % NGT][:],
                                   transpose=True)
                    return ftp2

                for pr in range(2):
                    _cast_pair(pr)
                ftps[0] = _transposes(0)
                NDV = 9   # SoS chunks on DVE; rest on Scalar
                for pr in range(NP):
                    if pr + 2 < NP:
                        _cast_pair(pr + 2)
                    if pr + 1 < NP:
                        ftps[pr + 1] = _transposes(pr + 1)
                    ftp2 = ftps.pop(pr)
                    for tt in range(2):
                        t = 2 * pr + tt
                        tsl = slice(t * NCH, (t + 1) * NCH)
                        ss = ps.tile([128, NCH], f32, tag="ss",
                                     name=f"ss_{t}")
                        for j in range(NCH):
                            src = ftp2[:, :, tt, j, :]
                            if j < NDV:
                                nc.vector.scalar_tensor_tensor(
                                    out=sq_v[:], in0=src, scalar=1.0,
                                    in1=src, op0=ALU.mult, op1=ALU.mult,
                                    accum_out=ss[:, j:j + 1])
                            else:
                                nc.scalar.activation(
                                    sq_s[:], src, AF.Square,
                                    accum_out=ss[:, j:j + 1])
                        srt = ps.tile([128, NCH], f32, tag="srt",
                                      name=f"srt_{t}")
                        nc.scalar.sqrt(srt[:], ss[:])
                        nc.vector.reciprocal(invr_all[:, tsl], srt[:])

                        posw = ps.tile([128, NCH, K], bf16, tag="posw",
                                       name=f"posw_{t}")
                        nc.vector.tensor_mul(
                            posw[:], posT16[:, tsl, 0:K],
                            invr_all[:, tsl].unsqueeze(2).broadcast_to(
                                [128, NCH, K]))
                        nc.vector.tensor_reduce(
                            npos_cols[:, tsl], posT16[:, tsl, 0:K],
                            axis=AX.X, op=ALU.add)

                        for j in range(NCH):
                            nc.tensor.matmul(
                                k0_ps[:], posw[:, j, :],
                                ftp2[:, :, tt, j, :],
                                start=(t == 0 and j == 0),
                                stop=(t == NT - 1 and j == NCH - 1),
                                skip_group_check=True)

                k0_sb = pp.tile([K, C], f32)
                nc.scalar.copy(k0_sb[:], k0_ps[:])

                # dummy matmuls keep the PE HAM-warm across the collective
                # gap (write garbage into k0_ps, already copied out)
                for w in range(NWARM):
                    nc.tensor.matmul(k0_ps[:], wsrc[:, 0:K], wsrc[:, 0:C],
                                     start=True, stop=True,
                                     skip_group_check=True)

            # ---------------- AllReduce k0 across 8 cores ----------------
            k0_loc = pd.tile([K, C], f32)
            k0_sum = pd.tile([K, C], f32)
            nc.sync.dma_start(k0_loc[:], k0_sb[:])
            nc.gpsimd.collective_compute(
                "AllReduce", ALU.add,
                ins=[k0_loc.opt()],
                outs=[k0_sum.opt()],
                replica_groups=[list(range(ncores))],
            )
            k0t = pp.tile([K, C], f32)
            nc.sync.dma_start(k0t[:], k0_sum[:])

            # k0ns = (k0 / max(||k0||, eps)) / tau, transposed to [c, 2, K]
            k0sq = pp.tile([K, C], f32)
            ssk = pp.tile([K, 1], f32)
            nc.scalar.activation(k0sq[:], k0t[:], AF.Square, accum_out=ssk[:])
            sk = pp.tile([K, 1], f32)
            nc.scalar.sqrt(sk[:], ssk[:])
            skm = pp.tile([K, 1], f32)
            nc.vector.tensor_scalar_max(skm[:], sk[:], EPS)
            invk = pp.tile([K, 1], f32)
            nc.vector.reciprocal(invk[:], skm[:])
            invks = pp.tile([K, 1], f32)
            nc.scalar.mul(invks[:], invk[:], 1.0 / TAU)
            # bf16 k0ns staged in the zero-padded 32-row tile, one tiny xbar
            # transpose gives k0n^T [c, 2, K] without touching PE/PSUM
            nc.vector.tensor_scalar_mul(gt16[0][0:K, 0:C], k0t[:], invks[:])
            k0nT16 = pp.tile([128, 2, KP], bf16)
            nc.sync.dma_start(k0nT16[:], gt16[0][:, 0:C], transpose=True)

            # dot partial: sum pos*z over this core's pixels
            #   = sum_kc (k0t*invks)[k,c] * k0_local[k,c]
            dotm = pp.tile([K, C], f32)
            nc.gpsimd.tensor_mul(dotm[:], k0t[:], k0_sb[:])
            dvec = pp.tile([K, 1], f32)
            nc.vector.reduce_sum(dvec[:], dotm[:], axis=AX.X)
            nc.vector.tensor_mul(lred[0:K, 2:3], dvec[:], invks[:])

            # ---------------- phase 2: logits, softmax denom, loss --------
            with tc.tile_pool(name="psB", bufs=2, space="PSUM") as psB:
                for t in range(NT):
                    pr, tt = t // 2, t % 2
                    lgA = psB.tile([K, 2, GP], f32, tag="lgA",
                                   name=f"lgA_{t}")
                    lgB = psB.tile([K, 2, GP], f32, tag="lgB",
                                   name=f"lgB_{t}")
                    lgs = [lgA[:, 0, :], lgA[:, 1, :], lgB[:, 0, :],
                           lgB[:, 1, :]]
                    for g in range(NG):
                        gsl = slice(g * GP, (g + 1) * GP)
                        nc.tensor.matmul(
                            lgs[g], k0nT16[:, 0, 0:K],
                            fa16[:, t // 2, 0, t % 2, gsl],
                            start=True, stop=False, skip_group_check=True)
                    for g in range(NG):
                        gsl = slice(g * GP, (g + 1) * GP)
                        nc.tensor.matmul(
                            lgs[g], k0nT16[:, 1, 0:K],
                            fa16[:, t // 2, 1, t % 2, gsl],
                            start=False, stop=True, skip_group_check=True)

                    # PSUM -> K-major bf16 staging (per tile), one xbar
                    # transpose + batched softmax denominator per pair
                    zs = gt16[pr % NGT]
                    zo = tt * TILE_PIX
                    nc.scalar.copy(zs[0:K, zo:zo + 2 * GP], lgA[:])
                    nc.vector.tensor_copy(zs[0:K, zo + 2 * GP:zo + 4 * GP],
                                          lgB[:])
                    if tt == 0:
                        continue
                    ptsl = slice(2 * pr * NCH, (2 * pr + 2) * NCH)
                    zT = p2.tile([128, 2 * NCH, KP], bf16, tag="zT",
                                 name=f"zT_{pr}")
                    teng = nc.sync if pr % 2 == 0 else nc.scalar
                    teng.dma_start(zT[:], zs[:], transpose=True)

                    ib = invr_all[:, ptsl].unsqueeze(2).broadcast_to(
                        [128, 2 * NCH, K])
                    y = p2.tile([128, 2 * NCH, K], f32, tag="y",
                                name=f"y_{pr}")
                    nc.gpsimd.tensor_mul(y[:], zT[:, :, 0:K], ib)
                    nc.scalar.activation(y[:], y[:], AF.Exp)
                    nc.vector.reduce_sum(s_all[:, ptsl], y[:], axis=AX.X)

                # deferred loss tail, batched over all 256 columns
                nc.scalar.activation(s_all[:], s_all[:], AF.Ln)
                nc.vector.reduce_sum(lred[:, 1:2], npos_cols[:], axis=AX.X)
                nc.vector.tensor_mul(npos_cols[:], npos_cols[:], s_all[:])
                nc.vector.reduce_sum(lred[:, 0:1], npos_cols[:], axis=AX.X)

                lfin = pp.tile([128, 4], f32)
                nc.gpsimd.partition_all_reduce(
                    lfin[:, 0:3], lred[:, 0:3], channels=128,
                    reduce_op=bass_isa.ReduceOp.add)
                nc.sync.dma_start(out_part[:], lfin[0:1, 0:3])

    nc.compile()
    return nc


def kernel(feat: np.ndarray, gt: np.ndarray) -> np.ndarray:
    from concourse.bass_utils import run_bass_kernel_spmd

    if "nc" not in _CACHE:
        _CACHE["nc"] = _build_nc()
    nc = _CACHE["nc"]

    feat_r = np.ascontiguousarray(feat, dtype=np.float32).reshape(B, C, HW)
    gt_r = np.ascontiguousarray(gt, dtype=np.float32).reshape(B, K, HW)
    per_batch = NCORES // B                       # 2 shards per image
    span = HW // per_batch                        # 32768
    in_maps = []
    for m in range(NCORES):
        b, lo = m // per_batch, (m % per_batch) * span
        in_maps.append({
            "feat_s": np.ascontiguousarray(feat_r[b, :, lo:lo + span]),
            "gt_s": np.ascontiguousarray(gt_r[b, :, lo:lo + span]),
        })

    res = run_bass_kernel_spmd(nc, in_maps, list(range(NCORES)))
    _CACHE["last_results"] = res
    parts = np.stack([r["part"].reshape(3) for r in res.results])
    nll_sum = float(np.sum(parts[:, 0].astype(np.float64)))
    num_pos = float(np.sum(parts[:, 1].astype(np.float64)))
    dot_sum = float(np.sum(parts[:, 2].astype(np.float64)))
    return np.asarray((nll_sum - dot_sum) / num_pos, dtype=np.float32)


# revision 9
# speedup vs baseline: 1.1889x; 1.0800x over previous
"""GroupContrastLoss on 8 trn2 NeuronCores via Bass/Tile.

Math (reference):
  pos   = (gt == 1)                              [B,K,H,W]
  fnorm = feat / max(||feat||_C, eps)            per-pixel L2 over C
  k0    = einsum('bkhw,bchw->kc', pos, fnorm)    [K,C]   (global sum!)
  k0n   = k0 / max(||k0||_C, eps)
  logits= einsum('kc,bchw->bkhw', k0n, fnorm)/tau
  loss  = -sum(pos * log_softmax(logits, k)) / sum(pos)

Key identity: sum_pix pos*logits = <k0n/tau, k0_local> per core, since
k0_local is by definition sum_pix pos*invr*feat.  This removes the whole
pos*z pixel path from phase 2 (no posT16 read, no second reduce).

Sharding: pixels (b, hw) split into 8 contiguous shards (2 per batch image,
32768 pixels each).  Per-core partial k0 [19,256] is AllReduced on-device
between the phases; scalar partials (sum npos*lse, num_pos, <k0ns,k0_loc>)
are combined on host.

Phase 1: feat is read from HBM exactly once via gpsimd casting DMAs (f32 in
DRAM -> bf16 in SBUF) into a channel-half-OUTER stash fa16[128, NP, 2, 2, 2048]
so each cast descriptor is one contiguous 8KB run per partition (128 descs
per dma_start instead of 256 -- Q7 descriptor generation was the cast-path
concurrency limiter).  Pixel-major feat/gt come from per-pair 2MB xbar DMA
transposes; DMA_TRANSPOSE blocks its issuing queue for the transfer
duration, so transposes alternate between the two HWDGE queues
(sync/scalar).  Per-pixel inv-norms come from chunked fused square+
accumulate ops (DVE scalar_tensor_tensor + Scalar Square+accum_out); invr
folds into the transposed gt mask (posw) for the k0 accumulation (16 bf16
matmuls per tile, PE otherwise free in phase 1).

Between phases a chain of dummy matmuls keeps the PE HAM-warm across the
collective gap so phase-2 matmuls run at 2.4GHz instead of 1.2GHz.

Phase 2 computes logits as [K,512] bf16 matmuls (k0nT stationary) straight
from the contiguous fa16 slices, stages PSUM->SBUF as bf16 K-major in the
gt16 buffers, one xbar transpose per pair back to pixel-major, then only
the exp/sum softmax denominator.  Ln and the loss combination happen in one
batched tail (EXP and LN live in different hw activation tables; per-tile
Ln would thrash table loads).
"""

import numpy as np

TAU = 0.07
EPS = 1e-12
B, C, H, W, K = 4, 256, 256, 256, 19
HW = H * W
NCORES = 8
SHARD = B * HW // NCORES        # 32768 pixels per core
TILE_PIX = 2048                 # pixels per tile iteration
NCH = TILE_PIX // 128           # 16 chunks of 128 pixels
NT = SHARD // TILE_PIX          # 16 tile iterations
NP = NT // 2                    # 8 tile pairs
CH = C // 2                     # 128, feat channel half
KP = 32                         # classes padded to 32 for DMA transpose
GP = 512                        # pixels per logit matmul group
NG = TILE_PIX // GP             # 4 groups per tile
NWARM = 220                     # dummy matmuls bridging the collective gap

_CACHE = {}


def _build_nc(ncores=NCORES):
    import concourse.bass as bass
    import concourse.bass_isa as bass_isa
    import concourse.bacc as bacc
    import concourse.mybir as mybir
    from concourse import tile, masks

    f32 = mybir.dt.float32
    bf16 = mybir.dt.bfloat16
    AX = mybir.AxisListType
    AF = mybir.ActivationFunctionType
    ALU = mybir.AluOpType

    nc = bacc.Bacc("TRN2", target_bir_lowering=False, debug=False,
                   num_devices=ncores)

    feat_in = nc.dram_tensor("feat_s", [C, SHARD], f32, kind="ExternalInput")
    gt_in = nc.dram_tensor("gt_s", [K, SHARD], f32, kind="ExternalInput")
    out_part = nc.dram_tensor("part", [1, 3], f32, kind="ExternalOutput")

    with tile.TileContext(nc) as tc:
        with (
            tc.tile_pool(name="persist", bufs=1) as pp,
            tc.tile_pool(name="ft", bufs=2) as pft,
            tc.tile_pool(name="small", bufs=2) as ps,
            tc.tile_pool(name="p2", bufs=1) as p2,
            tc.tile_pool(name="dram", bufs=1, space="DRAM") as pd,
        ):
            # persistent stashes
            # channel-major bf16 feat, channel-half OUTERMOST so each cast
            # DMA's per-partition run is one contiguous 8KB block
            fa16 = pp.tile([128, NP, 2, 2, TILE_PIX], bf16)
            posT16 = pp.tile([128, NT * NCH, KP], bf16)   # gt transposed
            invr_all = pp.tile([128, NT * NCH], f32)
            npos_cols = pp.tile([128, NT * NCH], f32)
            s_all = pp.tile([128, NT * NCH], f32)
            sq_v = pp.tile([128, C], bf16)                # SoS scratch outs
            sq_s = pp.tile([128, C], bf16)
            wsrc = pp.tile([128, GP], bf16)               # warmup mm source
            lred = pp.tile([128, 4], f32)                 # loss partials

            nc.vector.memset(wsrc[:], 0.0)
            nc.gpsimd.memset(lred[:], 0.0)

            # pair-sized K-major staging, shared by phase 1 (gt) and
            # phase 2 (z); rows K:KP zeroed once, never rewritten
            NGT = 2
            gt16 = [pp.tile([KP, 2 * TILE_PIX], bf16, name=f"gt16_{i}")
                    for i in range(NGT)]
            nc.vector.memset(gt16[0][:], 0.0)
            nc.gpsimd.memset(gt16[1][:], 0.0)

            # ---------------- phase 1: k0 accumulation ----------------
            with tc.tile_pool(name="psA", bufs=1, space="PSUM") as psA:
                k0_ps = psA.tile([K, C], f32)
                ftps = {}

                def _gt_cast(pr):
                    psl = slice(2 * pr * TILE_PIX, (2 * pr + 2) * TILE_PIX)
                    nc.gpsimd.dma_start(gt16[pr % NGT][0:K, :],
                                        gt_in[:, psl])

                def _feat_cast(pr):
                    psl = slice(2 * pr * TILE_PIX, (2 * pr + 2) * TILE_PIX)
                    nc.gpsimd.dma_start(fa16[:, pr, 0, :, :],
                                        feat_in[0:CH, psl])
                    nc.gpsimd.dma_start(fa16[:, pr, 1, :, :],
                                        feat_in[CH:C, psl])

                def _transposes(pr):
                    ptsl = slice(2 * pr * NCH, (2 * pr + 2) * NCH)
                    ftp2 = pft.tile([128, 2, 2, NCH, 128], bf16, tag="ftp",
                                    name=f"ftp_{pr}")
                    feng = nc.sync if pr % 2 == 0 else nc.scalar
                    geng = nc.scalar if pr % 2 == 0 else nc.sync
                    feng.dma_start(ftp2[:], fa16[:, pr, :, :, :],
                                   transpose=True)
                    geng.dma_start(posT16[:, ptsl, :], gt16[pr % NGT][:],
                                   transpose=True)
                    return ftp2

                # gt casts for the first two pairs, then ALL feat casts
                # upfront -- fa16 slices are persistent, nothing blocks
                # them, and a deep SWDGE queue keeps all 16 SDMA engines fed
                for pr in range(2):
                    _gt_cast(pr)
                for pr in range(NP):
                    _feat_cast(pr)
                ftps[0] = _transposes(0)
                NDV, NSC = 9, 7   # SoS chunks: DVE 9, Scalar 7
                for pr in range(NP):
                    if pr + 2 < NP:
                        _gt_cast(pr + 2)
                    if pr + 1 < NP:
                        ftps[pr + 1] = _transposes(pr + 1)
                    ftp2 = ftps.pop(pr)
                    for tt in range(2):
                        t = 2 * pr + tt
                        tsl = slice(t * NCH, (t + 1) * NCH)
                        ss = ps.tile([128, NCH], f32, tag="ss",
                                     name=f"ss_{t}")
                        for j in range(NCH):
                            src = ftp2[:, :, tt, j, :]
                            if j < NDV:
                                nc.vector.scalar_tensor_tensor(
                                    out=sq_v[:], in0=src, scalar=1.0,
                                    in1=src, op0=ALU.mult, op1=ALU.mult,
                                    accum_out=ss[:, j:j + 1])
                            else:
                                nc.scalar.activation(
                                    sq_s[:], src, AF.Square,
                                    accum_out=ss[:, j:j + 1])
                        srt = ps.tile([128, NCH], f32, tag="srt",
                                      name=f"srt_{t}")
                        nc.scalar.sqrt(srt[:], ss[:])
                        nc.vector.reciprocal(invr_all[:, tsl], srt[:])

                        posw = ps.tile([128, NCH, K], bf16, tag="posw",
                                       name=f"posw_{t}")
                        nc.gpsimd.tensor_mul(
                            posw[:], posT16[:, tsl, 0:K],
                            invr_all[:, tsl].unsqueeze(2).broadcast_to(
                                [128, NCH, K]))
                        nc.vector.tensor_reduce(
                            npos_cols[:, tsl], posT16[:, tsl, 0:K],
                            axis=AX.X, op=ALU.add)

                        for j in range(NCH):
                            nc.tensor.matmul(
                                k0_ps[:], posw[:, j, :],
                                ftp2[:, :, tt, j, :],
                                start=(t == 0 and j == 0),
                                stop=(t == NT - 1 and j == NCH - 1),
                                skip_group_check=True)

                k0_sb = pp.tile([K, C], f32)
                nc.scalar.copy(k0_sb[:], k0_ps[:])

                # dummy matmuls keep the PE HAM-warm across the collective
                # gap (write garbage into k0_ps, already copied out)
                for w in range(NWARM):
                    nc.tensor.matmul(k0_ps[:], wsrc[:, 0:K], wsrc[:, 0:C],
                                     start=True, stop=True,
                                     skip_group_check=True)

            # ---------------- AllReduce k0 across 8 cores ----------------
            k0_loc = pd.tile([K, C], f32)
            k0_sum = pd.tile([K, C], f32)
            nc.sync.dma_start(k0_loc[:], k0_sb[:])
            nc.gpsimd.collective_compute(
                "AllReduce", ALU.add,
                ins=[k0_loc.opt()],
                outs=[k0_sum.opt()],
                replica_groups=[list(range(ncores))],
            )
            k0t = pp.tile([K, C], f32)
            nc.sync.dma_start(k0t[:], k0_sum[:])

            # k0ns = (k0 / max(||k0||, eps)) / tau, transposed to [c, 2, K]
            k0sq = pp.tile([K, C], f32)
            ssk = pp.tile([K, 1], f32)
            nc.scalar.activation(k0sq[:], k0t[:], AF.Square, accum_out=ssk[:])
            sk = pp.tile([K, 1], f32)
            nc.scalar.sqrt(sk[:], ssk[:])
            skm = pp.tile([K, 1], f32)
            nc.vector.tensor_scalar_max(skm[:], sk[:], EPS)
            invk = pp.tile([K, 1], f32)
            nc.vector.reciprocal(invk[:], skm[:])
            invks = pp.tile([K, 1], f32)
            nc.scalar.mul(invks[:], invk[:], 1.0 / TAU)
            # bf16 k0ns staged in the zero-padded 32-row tile, one tiny xbar
            # transpose gives k0n^T [c, 2, K] without touching PE/PSUM
            nc.vector.tensor_scalar_mul(gt16[0][0:K, 0:C], k0t[:], invks[:])
            k0nT16 = pp.tile([128, 2, KP], bf16)
            nc.sync.dma_start(k0nT16[:], gt16[0][:, 0:C], transpose=True)

            # dot partial: sum pos*z over this core's pixels
            #   = sum_kc (k0t*invks)[k,c] * k0_local[k,c]
            dotm = pp.tile([K, C], f32)
            nc.gpsimd.tensor_mul(dotm[:], k0t[:], k0_sb[:])
            dvec = pp.tile([K, 1], f32)
            nc.vector.reduce_sum(dvec[:], dotm[:], axis=AX.X)
            nc.vector.tensor_mul(lred[0:K, 2:3], dvec[:], invks[:])

            # ---------------- phase 2: logits, softmax denom, loss --------
            with tc.tile_pool(name="psB", bufs=2, space="PSUM") as psB:
                for t in range(NT):
                    pr, tt = t // 2, t % 2
                    lgA = psB.tile([K, 2, GP], f32, tag="lgA",
                                   name=f"lgA_{t}")
                    lgB = psB.tile([K, 2, GP], f32, tag="lgB",
                                   name=f"lgB_{t}")
                    lgs = [lgA[:, 0, :], lgA[:, 1, :], lgB[:, 0, :],
                           lgB[:, 1, :]]
                    for g in range(NG):
                        gsl = slice(g * GP, (g + 1) * GP)
                        nc.tensor.matmul(
                            lgs[g], k0nT16[:, 0, 0:K],
                            fa16[:, t // 2, 0, t % 2, gsl],
                            start=True, stop=False, skip_group_check=True)
                    for g in range(NG):
                        gsl = slice(g * GP, (g + 1) * GP)
                        nc.tensor.matmul(
                            lgs[g], k0nT16[:, 1, 0:K],
                            fa16[:, t // 2, 1, t % 2, gsl],
                            start=False, stop=True, skip_group_check=True)

                    # PSUM -> K-major bf16 staging (per tile), one xbar
                    # transpose + batched softmax denominator per pair
                    zs = gt16[pr % NGT]
                    zo = tt * TILE_PIX
                    nc.scalar.copy(zs[0:K, zo:zo + 2 * GP], lgA[:])
                    nc.vector.tensor_copy(zs[0:K, zo + 2 * GP:zo + 4 * GP],
                                          lgB[:])
                    if tt == 0:
                        continue
                    ptsl = slice(2 * pr * NCH, (2 * pr + 2) * NCH)
                    zT = p2.tile([128, 2 * NCH, KP], bf16, tag="zT",
                                 name=f"zT_{pr}")
                    teng = nc.sync if pr % 2 == 0 else nc.scalar
                    teng.dma_start(zT[:], zs[:], transpose=True)

                    ib = invr_all[:, ptsl].unsqueeze(2).broadcast_to(
                        [128, 2 * NCH, K])
                    y = p2.tile([128, 2 * NCH, K], f32, tag="y",
                                name=f"y_{pr}")
                    nc.gpsimd.tensor_mul(y[:], zT[:, :, 0:K], ib)
                    nc.scalar.activation(y[:], y[:], AF.Exp)
                    nc.vector.reduce_sum(s_all[:, ptsl], y[:], axis=AX.X)

                # deferred loss tail, batched over all 256 columns
                nc.scalar.activation(s_all[:], s_all[:], AF.Ln)
                nc.vector.reduce_sum(lred[:, 1:2], npos_cols[:], axis=AX.X)
                nc.vector.tensor_mul(npos_cols[:], npos_cols[:], s_all[:])
                nc.vector.reduce_sum(lred[:, 0:1], npos_cols[:], axis=AX.X)

                lfin = pp.tile([128, 4], f32)
                nc.gpsimd.partition_all_reduce(
                    lfin[:, 0:3], lred[:, 0:3], channels=128,
                    reduce_op=bass_isa.ReduceOp.add)
                nc.sync.dma_start(out_part[:], lfin[0:1, 0:3])

    nc.compile()
    return nc


def kernel(feat: np.ndarray, gt: np.ndarray) -> np.ndarray:
    from concourse.bass_utils import run_bass_kernel_spmd

    if "nc" not in _CACHE:
        _CACHE["nc"] = _build_nc()
    nc = _CACHE["nc"]

    feat_r = np.ascontiguousarray(feat, dtype=np.float32).reshape(B, C, HW)
    gt_r = np.ascontiguousarray(gt, dtype=np.float32).reshape(B, K, HW)
    per_batch = NCORES // B                       # 2 shards per image
    span = HW // per_batch                        # 32768
    in_maps = []
    for m in range(NCORES):
        b, lo = m // per_batch, (m % per_batch) * span
        in_maps.append({
            "feat_s": np.ascontiguousarray(feat_r[b, :, lo:lo + span]),
            "gt_s": np.ascontiguousarray(gt_r[b, :, lo:lo + span]),
        })

    res = run_bass_kernel_spmd(nc, in_maps, list(range(NCORES)))
    _CACHE["last_results"] = res
    parts = np.stack([r["part"].reshape(3) for r in res.results])
    nll_sum = float(np.sum(parts[:, 0].astype(np.float64)))
    num_pos = float(np.sum(parts[:, 1].astype(np.float64)))
    dot_sum = float(np.sum(parts[:, 2].astype(np.float64)))
    return np.asarray((nll_sum - dot_sum) / num_pos, dtype=np.float32)


# revision 10
# speedup vs baseline: 1.2337x; 1.0377x over previous
"""GroupContrastLoss on 8 trn2 NeuronCores via Bass/Tile.

Math (reference):
  pos   = (gt == 1)                              [B,K,H,W]
  fnorm = feat / max(||feat||_C, eps)            per-pixel L2 over C
  k0    = einsum('bkhw,bchw->kc', pos, fnorm)    [K,C]   (global sum!)
  k0n   = k0 / max(||k0||_C, eps)
  logits= einsum('kc,bchw->bkhw', k0n, fnorm)/tau
  loss  = -sum(pos * log_softmax(logits, k)) / sum(pos)

Key identity: sum_pix pos*logits = <k0n/tau, k0_local> per core, since
k0_local is by definition sum_pix pos*invr*feat.  This removes the whole
pos*z pixel path from phase 2 (no posT16 read, no second reduce).

Sharding: pixels (b, hw) split into 8 contiguous shards (2 per batch image,
32768 pixels each).  Per-core partial k0 [19,256] is AllReduced on-device
between the phases; scalar partials (sum npos*lse, num_pos, <k0ns,k0_loc>)
are combined on host.

Phase 1: feat is read from HBM exactly once via gpsimd casting DMAs (f32 in
DRAM -> bf16 in SBUF) into a channel-half-OUTER stash fa16[128, NP, 2, 2, 2048]
so each cast descriptor is one contiguous 8KB run per partition (128 descs
per dma_start instead of 256 -- Q7 descriptor generation was the cast-path
concurrency limiter).  Pixel-major feat/gt come from per-pair 2MB xbar DMA
transposes; DMA_TRANSPOSE blocks its issuing queue for the transfer
duration, so transposes alternate between the two HWDGE queues
(sync/scalar).  Per-pixel inv-norms come from chunked fused square+
accumulate ops (DVE scalar_tensor_tensor + Scalar Square+accum_out); invr
folds into the transposed gt mask (posw) for the k0 accumulation (16 bf16
matmuls per tile, PE otherwise free in phase 1).

Between phases a chain of dummy matmuls keeps the PE HAM-warm across the
collective gap so phase-2 matmuls run at 2.4GHz instead of 1.2GHz.

Phase 2 computes logits as [K,512] bf16 matmuls (k0nT stationary) straight
from the contiguous fa16 slices, stages PSUM->SBUF as bf16 K-major in the
gt16 buffers, one xbar transpose per pair back to pixel-major, then only
the exp/sum softmax denominator.  Ln and the loss combination happen in one
batched tail (EXP and LN live in different hw activation tables; per-tile
Ln would thrash table loads).
"""

import numpy as np

TAU = 0.07
EPS = 1e-12
B, C, H, W, K = 4, 256, 256, 256, 19
HW = H * W
NCORES = 8
SHARD = B * HW // NCORES        # 32768 pixels per core
TILE_PIX = 2048                 # pixels per tile iteration
NCH = TILE_PIX // 128           # 16 chunks of 128 pixels
NT = SHARD // TILE_PIX          # 16 tile iterations
NP = NT // 2                    # 8 tile pairs
CH = C // 2                     # 128, feat channel half
KP = 32                         # classes padded to 32 for DMA transpose
GP = 512                        # pixels per logit matmul group
NG = TILE_PIX // GP             # 4 groups per tile
NWARM = 220                     # dummy matmuls bridging the collective gap

_CACHE = {}


def _build_nc(ncores=NCORES):
    import concourse.bass as bass
    import concourse.bass_isa as bass_isa
    import concourse.bacc as bacc
    import concourse.mybir as mybir
    from concourse import tile, masks

    f32 = mybir.dt.float32
    bf16 = mybir.dt.bfloat16
    AX = mybir.AxisListType
    AF = mybir.ActivationFunctionType
    ALU = mybir.AluOpType

    nc = bacc.Bacc("TRN2", target_bir_lowering=False, debug=False,
                   num_devices=ncores)

    feat_in = nc.dram_tensor("feat_s", [C, SHARD], f32, kind="ExternalInput")
    gt_in = nc.dram_tensor("gt_s", [K, SHARD], f32, kind="ExternalInput")
    out_part = nc.dram_tensor("part", [1, 3], f32, kind="ExternalOutput")

    with tile.TileContext(nc) as tc:
        with (
            tc.tile_pool(name="persist", bufs=1) as pp,
            tc.tile_pool(name="ft", bufs=2) as pft,
            tc.tile_pool(name="small", bufs=2) as ps,
            tc.tile_pool(name="p2", bufs=1) as p2,
            tc.tile_pool(name="dram", bufs=1, space="DRAM") as pd,
        ):
            # persistent stashes
            # channel-major bf16 feat, channel-half OUTERMOST so each cast
            # DMA's per-partition run is one contiguous 8KB block
            fa16 = pp.tile([128, NP, 2, 2, TILE_PIX], bf16)
            posT16 = pp.tile([128, NT * NCH, KP], bf16)   # gt transposed
            invr_all = pp.tile([128, NT * NCH], f32)
            npos_cols = pp.tile([128, NT * NCH], f32)
            s_all = pp.tile([128, NT * NCH], f32)
            sq_v = pp.tile([128, C], bf16)                # SoS scratch outs
            sq_s = pp.tile([128, C], bf16)
            wsrc = pp.tile([128, GP], bf16)               # warmup mm source
            lred = pp.tile([128, 4], f32)                 # loss partials

            nc.vector.memset(wsrc[:], 0.0)
            nc.gpsimd.memset(lred[:], 0.0)

            # pair-sized K-major staging, shared by phase 1 (gt) and
            # phase 2 (z); rows K:KP zeroed once, never rewritten
            NGT = 2
            gt16 = [pp.tile([KP, 2 * TILE_PIX], bf16, name=f"gt16_{i}")
                    for i in range(NGT)]
            nc.vector.memset(gt16[0][:], 0.0)
            nc.gpsimd.memset(gt16[1][:], 0.0)

            # ---------------- phase 1: k0 accumulation ----------------
            with tc.tile_pool(name="psA", bufs=1, space="PSUM") as psA:
                k0_ps = psA.tile([K, C], f32)
                ftps = {}

                def _gt_cast(pr):
                    psl = slice(2 * pr * TILE_PIX, (2 * pr + 2) * TILE_PIX)
                    nc.gpsimd.dma_start(gt16[pr % NGT][0:K, :],
                                        gt_in[:, psl])

                def _feat_cast(pr):
                    psl = slice(2 * pr * TILE_PIX, (2 * pr + 2) * TILE_PIX)
                    nc.gpsimd.dma_start(fa16[:, pr, 0, :, :],
                                        feat_in[0:CH, psl])
                    nc.gpsimd.dma_start(fa16[:, pr, 1, :, :],
                                        feat_in[CH:C, psl])

                def _transposes(pr):
                    ptsl = slice(2 * pr * NCH, (2 * pr + 2) * NCH)
                    ftp2 = pft.tile([128, 2, 2, NCH, 128], bf16, tag="ftp",
                                    name=f"ftp_{pr}")
                    feng = nc.sync if pr % 2 == 0 else nc.scalar
                    geng = nc.scalar if pr % 2 == 0 else nc.sync
                    feng.dma_start(ftp2[:], fa16[:, pr, :, :, :],
                                   transpose=True)
                    geng.dma_start(posT16[:, ptsl, :], gt16[pr % NGT][:],
                                   transpose=True)
                    return ftp2

                # gt casts for the first two pairs, then ALL feat casts
                # upfront -- fa16 slices are persistent, nothing blocks
                # them, and a deep SWDGE queue keeps all 16 SDMA engines fed
                for pr in range(2):
                    _gt_cast(pr)
                for pr in range(NP):
                    _feat_cast(pr)
                ftps[0] = _transposes(0)
                NDV, NSC = 9, 7   # SoS chunks: DVE 9, Scalar 7
                for pr in range(NP):
                    if pr + 2 < NP:
                        _gt_cast(pr + 2)
                    if pr + 1 < NP:
                        ftps[pr + 1] = _transposes(pr + 1)
                    ftp2 = ftps.pop(pr)
                    for tt in range(2):
                        t = 2 * pr + tt
                        tsl = slice(t * NCH, (t + 1) * NCH)
                        ss = ps.tile([128, NCH], f32, tag="ss",
                                     name=f"ss_{t}")
                        for j in range(NCH):
                            src = ftp2[:, :, tt, j, :]
                            if j < NDV:
                                nc.vector.scalar_tensor_tensor(
                                    out=sq_v[:], in0=src, scalar=1.0,
                                    in1=src, op0=ALU.mult, op1=ALU.mult,
                                    accum_out=ss[:, j:j + 1])
                            else:
                                nc.scalar.activation(
                                    sq_s[:], src, AF.Square,
                                    accum_out=ss[:, j:j + 1])
                        srt = ps.tile([128, NCH], f32, tag="srt",
                                      name=f"srt_{t}")
                        nc.scalar.sqrt(srt[:], ss[:])
                        nc.vector.reciprocal(invr_all[:, tsl], srt[:])

                        posw = ps.tile([128, NCH, K], bf16, tag="posw",
                                       name=f"posw_{t}")
                        nc.vector.tensor_mul(
                            posw[:], posT16[:, tsl, 0:K],
                            invr_all[:, tsl].unsqueeze(2).broadcast_to(
                                [128, NCH, K]))
                        nc.vector.tensor_reduce(
                            npos_cols[:, tsl], posT16[:, tsl, 0:K],
                            axis=AX.X, op=ALU.add)

                        for j in range(NCH):
                            nc.tensor.matmul(
                                k0_ps[:], posw[:, j, :],
                                ftp2[:, :, tt, j, :],
                                start=(t == 0 and j == 0),
                                stop=(t == NT - 1 and j == NCH - 1),
                                skip_group_check=True)

                k0_sb = pp.tile([K, C], f32)
                nc.scalar.copy(k0_sb[:], k0_ps[:])

                # dummy matmuls keep the PE HAM-warm across the collective
                # gap (write garbage into k0_ps, already copied out)
                for w in range(NWARM):
                    nc.tensor.matmul(k0_ps[:], wsrc[:, 0:K], wsrc[:, 0:C],
                                     start=True, stop=True,
                                     skip_group_check=True)

            # ---------------- AllReduce k0 across 8 cores ----------------
            k0_loc = pd.tile([K, C], f32)
            k0_sum = pd.tile([K, C], f32)
            nc.sync.dma_start(k0_loc[:], k0_sb[:])
            nc.gpsimd.collective_compute(
                "AllReduce", ALU.add,
                ins=[k0_loc.opt()],
                outs=[k0_sum.opt()],
                replica_groups=[list(range(ncores))],
            )
            k0t = pp.tile([K, C], f32)
            nc.sync.dma_start(k0t[:], k0_sum[:])

            # k0ns = (k0 / max(||k0||, eps)) / tau, transposed to [c, 2, K]
            k0sq = pp.tile([K, C], f32)
            ssk = pp.tile([K, 1], f32)
            nc.scalar.activation(k0sq[:], k0t[:], AF.Square, accum_out=ssk[:])
            sk = pp.tile([K, 1], f32)
            nc.scalar.sqrt(sk[:], ssk[:])
            skm = pp.tile([K, 1], f32)
            nc.vector.tensor_scalar_max(skm[:], sk[:], EPS)
            invk = pp.tile([K, 1], f32)
            nc.vector.reciprocal(invk[:], skm[:])
            invks = pp.tile([K, 1], f32)
            nc.scalar.mul(invks[:], invk[:], 1.0 / TAU)
            # bf16 k0ns staged in the zero-padded 32-row tile, one tiny xbar
            # transpose gives k0n^T [c, 2, K] without touching PE/PSUM
            nc.vector.tensor_scalar_mul(gt16[0][0:K, 0:C], k0t[:], invks[:])
            k0nT16 = pp.tile([128, 2, KP], bf16)
            nc.sync.dma_start(k0nT16[:], gt16[0][:, 0:C], transpose=True)

            # dot partial: sum pos*z over this core's pixels
            #   = sum_kc (k0t*invks)[k,c] * k0_local[k,c]
            dotm = pp.tile([K, C], f32)
            nc.gpsimd.tensor_mul(dotm[:], k0t[:], k0_sb[:])
            dvec = pp.tile([K, 1], f32)
            nc.vector.reduce_sum(dvec[:], dotm[:], axis=AX.X)
            nc.vector.tensor_mul(lred[0:K, 2:3], dvec[:], invks[:])

            # ---------------- phase 2: logits, softmax denom, loss --------
            with tc.tile_pool(name="psB", bufs=2, space="PSUM") as psB:
                for t in range(NT):
                    pr, tt = t // 2, t % 2
                    lgA = psB.tile([K, 2, GP], f32, tag="lgA",
                                   name=f"lgA_{t}")
                    lgB = psB.tile([K, 2, GP], f32, tag="lgB",
                                   name=f"lgB_{t}")
                    lgs = [lgA[:, 0, :], lgA[:, 1, :], lgB[:, 0, :],
                           lgB[:, 1, :]]
                    for g in range(NG):
                        gsl = slice(g * GP, (g + 1) * GP)
                        nc.tensor.matmul(
                            lgs[g], k0nT16[:, 0, 0:K],
                            fa16[:, t // 2, 0, t % 2, gsl],
                            start=True, stop=False, skip_group_check=True)
                    for g in range(NG):
                        gsl = slice(g * GP, (g + 1) * GP)
                        nc.tensor.matmul(
                            lgs[g], k0nT16[:, 1, 0:K],
                            fa16[:, t // 2, 1, t % 2, gsl],
                            start=False, stop=True, skip_group_check=True)

                    # PSUM -> K-major bf16 staging (per tile), one xbar
                    # transpose + batched softmax denominator per pair
                    zs = gt16[pr % NGT]
                    zo = tt * TILE_PIX
                    nc.scalar.copy(zs[0:K, zo:zo + 2 * GP], lgA[:])
                    nc.vector.tensor_copy(zs[0:K, zo + 2 * GP:zo + 4 * GP],
                                          lgB[:])
                    if tt == 0:
                        continue
                    ptsl = slice(2 * pr * NCH, (2 * pr + 2) * NCH)
                    zT = p2.tile([128, 2 * NCH, KP], bf16, tag="zT",
                                 name=f"zT_{pr}")
                    teng = nc.sync if pr % 2 == 0 else nc.scalar
                    teng.dma_start(zT[:], zs[:], transpose=True)

                    ib = invr_all[:, ptsl].unsqueeze(2).broadcast_to(
                        [128, 2 * NCH, K])
                    y = p2.tile([128, 2 * NCH, K], f32, tag="y",
                                name=f"y_{pr}")
                    nc.gpsimd.tensor_mul(y[:], zT[:, :, 0:K], ib)
                    nc.scalar.activation(y[:], y[:], AF.Exp)
                    nc.vector.reduce_sum(s_all[:, ptsl], y[:], axis=AX.X)

                # deferred loss tail, batched over all 256 columns
                nc.scalar.activation(s_all[:], s_all[:], AF.Ln)
                nc.vector.reduce_sum(lred[:, 1:2], npos_cols[:], axis=AX.X)
                nc.vector.tensor_mul(npos_cols[:], npos_cols[:], s_all[:])
                nc.vector.reduce_sum(lred[:, 0:1], npos_cols[:], axis=AX.X)

                lfin = pp.tile([128, 4], f32)
                nc.gpsimd.partition_all_reduce(
                    lfin[:, 0:3], lred[:, 0:3], channels=128,
                    reduce_op=bass_isa.ReduceOp.add)
                nc.sync.dma_start(out_part[:], lfin[0:1, 0:3])

    nc.compile()
    return nc


def kernel(feat: np.ndarray, gt: np.ndarray) -> np.ndarray:
    from concourse.bass_utils import run_bass_kernel_spmd

    if "nc" not in _CACHE:
        _CACHE["nc"] = _build_nc()
    nc = _CACHE["nc"]

    feat_r = np.ascontiguousarray(feat, dtype=np.float32).reshape(B, C, HW)
    gt_r = np.ascontiguousarray(gt, dtype=np.float32).reshape(B, K, HW)
    per_batch = NCORES // B                       # 2 shards per image
    span = HW // per_batch                        # 32768
    in_maps = []
    for m in range(NCORES):
        b, lo = m // per_batch, (m % per_batch) * span
        in_maps.append({
            "feat_s": np.ascontiguousarray(feat_r[b, :, lo:lo + span]),
            "gt_s": np.ascontiguousarray(gt_r[b, :, lo:lo + span]),
        })

    res = run_bass_kernel_spmd(nc, in_maps, list(range(NCORES)))
    _CACHE["last_results"] = res
    parts = np.stack([r["part"].reshape(3) for r in res.results])
    nll_sum = float(np.sum(parts[:, 0].astype(np.float64)))
    num_pos = float(np.sum(parts[:, 1].astype(np.float64)))
    dot_sum = float(np.sum(parts[:, 2].astype(np.float64)))
    return np.asarray((nll_sum - dot_sum) / num_pos, dtype=np.float32)
